# revision 27
# baseline (speedup 1.0000x reference)
"""Guided filter (r=40, eps=1e-3) on 8 Trainium2 NeuronCores.

Sharding: pure data-parallel over the batch dim (8 batches -> 8 cores).
Each core processes 3 channel-images of 512x512.

Algorithm per image:
  box2d(x) as two banded matmuls on the TensorEngine (image chunk is the
  stationary operand, the 0/1 band matrix the moving operand; contraction
  runs over the partition dim so each pass both box-filters one axis and
  transposes the layout).  Everything on-chip is fp16: the band matrices
  carry fp16(1/n) directly (rel err ~3e-4, so no residual correction is
  needed anywhere), the host pre-packs fp16(I), fp16(p), fp16(I*p),
  fp16(I*I) into one stacked DRAM tensor.
  Stage B: each W-pass PSUM bank is freed by one fp16 stash copy; the
  elementwise chain then runs batched at [128,2048] (images 0,1) or
  [128,1024] halves (image 2, to keep the pipeline tail short) using
  DVE 2x fp16 tensor_tensor ops.  ~36 PE warmup matmuls at t=0 bridge
  the initial DMA wait and keep the PE HAM clock-gate warm.
"""

import os
import sys
import numpy as np
import ml_dtypes
from contextlib import ExitStack

sys.path.insert(0, "/opt/trn_rl_repo")

import concourse.bass as bass
import concourse.tile as tile
from concourse import bacc, mybir
from concourse.bass_utils import run_bass_kernel_spmd

F32 = mybir.dt.float32
F16 = mybir.dt.float16
ALU = mybir.AluOpType

R = 40
EPS = 1e-3
HW_ = 512
NB = 4  # 128-row blocks per axis
CH = 3  # channels per batch
NQ = 4  # stacked fp16 quantities: I, p, I*p, I*I
P = 128
NCORES = 8
N_WARMUP_MM = 16


_MUL_RECIP_OP = None


def _get_mul_recip_op():
    """Register a fused custom-DVE op: out = Src1 * recip_approx(Src0+C2),
    BITWISE_NOT exponent-flip seed + one inline Newton step (~0.4% rel err,
    one DVE pass instead of reciprocal + tensor_mul)."""
    global _MUL_RECIP_OP
    if _MUL_RECIP_OP is not None:
        return _MUL_RECIP_OP
    import re
    import concourse.dve_ops as dops
    from concourse.dve_spec import AluOp, Bin, C0, C1, C2, Spec, Src0, Src1

    name = "MUL_RECIP_EPS_GF"
    _x = Src0 + C2
    _not_x = Bin(AluOp.BITWISE_NOT, _x, _x)
    _y0 = _not_x * C0

    def _ref(in0, in1, c0, c1, c2):
        x = in0 + c2
        not_x = (~x.view(np.int32)).view(np.float32)
        y0 = not_x * c0
        return in1 * (y0 * (c1 - x * y0))

    op = dops.DveOp(
        name, Spec(body=Src1 * (_y0 * (C1 - _x * _y0)), reference=_ref),
        subdim=False, uops_sha={})
    dops.OPS.append(op)
    dops.CUSTOM_DVE_SPECS[name] = op.spec
    dops._SUB_OPCODE_FOR_NAME[name] = max(dops._SUB_OPCODE_FOR_NAME.values()) + 1
    for ver in ("v3", "v4"):
        try:
            op.compile(ver)
        except ValueError as e:
            m = re.search(r'uops_sha\["%s"\]="([0-9a-f]+)"' % ver, str(e))
            if not m:
                raise
            op.uops_sha[ver] = m.group(1)
            dops._COMPILE_CACHE.pop((name, ver), None)
            op.compile(ver)
    _MUL_RECIP_OP = op
    return op


def _band_range(c):
    n0 = max(0, P * c - R)
    n1 = min(HW_, P * c + P + R)
    return n0, n1


_BAND_OFF = []
_BAND_W = []
_off = 0
for _c in range(NB):
    _n0, _n1 = _band_range(_c)
    _BAND_OFF.append(_off)
    _BAND_W.append(_n1 - _n0)
    _off += _n1 - _n0
BAND_TOT = _off  # 792


def make_consts():
    idx = np.arange(HW_)
    n1d = (np.minimum(idx + R, HW_ - 1) - np.maximum(idx - R, 0) + 1).astype(np.float64)
    inv_n = 1.0 / n1d

    mask = (np.abs(idx[:, None] - idx[None, :]) <= R)
    band = (mask * inv_n[None, :]).astype(np.float16)
    # [512k, 512n] -> [128 kp, NB, 512] then pack only the band cols
    band = band.reshape(NB, P, HW_).transpose(1, 0, 2)
    cols = []
    for c in range(NB):
        n0, n1 = _band_range(c)
        cols.append(band[:, c, n0:n1])
    return {"band": np.ascontiguousarray(np.concatenate(cols, axis=1))}


def build_model():
    nc = bacc.Bacc("TRN2", target_bir_lowering=False, debug=False,
                   num_devices=NCORES)
    I_d = nc.dram_tensor("I", [CH, HW_, HW_], F32, kind="ExternalInput").ap()
    Q_d = nc.dram_tensor("Qf16", [CH, NQ, HW_, HW_], F16,
                         kind="ExternalInput").ap()
    band_d = nc.dram_tensor("band", [P, BAND_TOT], F16, kind="ExternalInput").ap()
    out_d = nc.dram_tensor("out", [CH, HW_, HW_], F32, kind="ExternalOutput").ap()

    with tile.TileContext(nc) as tc:
        with ExitStack() as ctx:
            build_kernel(ctx, tc, I_d, Q_d, out_d, band_d)
    nc.compile()
    return nc


def build_kernel(ctx, tc, I_d, Q_d, out_d, band_d):
    nc = tc.nc
    FW = NB * HW_    # 2048 free cols per quantity-image
    QW = NQ * FW     # 8192 free cols for the 4 stacked quantities
    HF = FW // 2     # 1024

    pQin = ctx.enter_context(tc.tile_pool(name="qin", bufs=2))
    pIf = ctx.enter_context(tc.tile_pool(name="If", bufs=3))
    consts = ctx.enter_context(tc.tile_pool(name="consts", bufs=1))
    pY = ctx.enter_context(tc.tile_pool(name="ymid", bufs=2))
    pAB = ctx.enter_context(tc.tile_pool(name="ab", bufs=2))
    pOut = ctx.enter_context(tc.tile_pool(name="outp", bufs=2))
    pS = ctx.enter_context(tc.tile_pool(name="stash", bufs=2))
    pS1 = ctx.enter_context(tc.tile_pool(name="chain", bufs=1))
    pV = ctx.enter_context(tc.tile_pool(name="psv", bufs=2, space="PSUM"))
    pQ = ctx.enter_context(tc.tile_pool(name="psq", bufs=1, space="PSUM"))

    # -- PE warmup: ~36 matmuls on a memset tile, no DMA dependencies. ----
    # Bridges the initial input-DMA wait and releases the HAM clock gate
    # before real matmuls arrive.
    wsrc = consts.tile([P, HW_], F16, tag="wsrc", name="wsrc")
    nc.gpsimd.memset(wsrc[:], 0.0)
    for k in range(N_WARMUP_MM):
        ps = pV.tile([P, 2 * HW_], F32, tag="ps")
        for i2 in range(2):
            nc.tensor.matmul(ps[:, i2 * HW_:i2 * HW_ + 256],
                             lhsT=wsrc[:, 0:P], rhs=wsrc[:, 0:256],
                             start=True, stop=True)

    def issue_dma(c, st):
        """Prefetch image c's inputs on the sync HWDGE queue: the guide
        image first (it gates the first V-pass), then p/Ip/II, then the
        f32 guide (only needed at stage C)."""
        Q = pQin.tile([P, QW], F16, tag="Qf16", name="Qf16")
        I_f = pIf.tile([P, FW], F32, tag="If", name="If")
        nc.sync.dma_start(
            Q[:, 0:2 * FW].rearrange("p (q hb w) -> p q hb w", q=2, w=HW_),
            Q_d[c, 0:2].rearrange("q (hb hp) w -> hp q hb w", hp=P))
        nc.sync.dma_start(
            Q[:, 2 * FW:QW].rearrange("p (q hb w) -> p q hb w", q=2, w=HW_),
            Q_d[c, 2:4].rearrange("q (hb hp) w -> hp q hb w", hp=P))
        nc.sync.dma_start(
            I_f[:].rearrange("p (hb w) -> p hb w", w=HW_),
            I_d[c].rearrange("(hb hp) w -> hp hb w", hp=P))
        st["Q"], st["I_f"] = Q, I_f

    band = consts.tile_from(band_d)

    def vpass(src, src_off, dst, engines):
        """One banded V-pass: src [h|w] fp16 -> dst [w|h] fp16 (box over
        partition axis + transpose). 16 MMs into two 2-bank PSUM tiles,
        each drained by one [128,1024] copy (engine per-half)."""
        for half in range(2):
            ps = pV.tile([P, 2 * HW_], F32, tag="ps")
            for i2 in range(2):
                i = 2 * half + i2
                for j in range(NB):
                    n0, n1 = _band_range(j)
                    o = src_off + j * HW_ + i * P
                    nc.tensor.matmul(
                        ps[:, i2 * HW_ + n0: i2 * HW_ + n1],
                        lhsT=src[:, o: o + P],
                        rhs=band[:, _BAND_OFF[j]: _BAND_OFF[j] + _BAND_W[j]],
                        start=(j == 0), stop=(j == NB - 1))
            d = dst[:, 2 * half * HW_: 2 * (half + 1) * HW_]
            if engines[half] == "dve":
                nc.vector.tensor_copy(d, ps[:])
            else:
                nc.scalar.copy(d, ps[:])

    def wpass_mm(src, q_tile, j):
        """W-direction banded MMs for output h-chunk j into q_tile."""
        for i in range(NB):
            m0, m1 = _band_range(i)
            nc.tensor.matmul(
                q_tile[:, m0:m1],
                lhsT=src[:, i * HW_ + j * P: i * HW_ + j * P + P],
                rhs=band[:, _BAND_OFF[i]: _BAND_OFF[i] + _BAND_W[i]],
                start=(i == 0), stop=(i == NB - 1))

    def stageA_units(st):
        """4 units: V-passes for I, p, Ip, II from the stacked input."""
        Q = st["Q"]
        for q, ytag in enumerate(("yI", "yp", "yIp", "yII")):
            y = pY.tile([P, FW], F16, tag=ytag, name=ytag)
            st[ytag] = y
            vpass(Q, q * FW, y, ["act", "act"])
            yield

    def stageB_units(st, nhalves):
        """4 j-units (W-passes + 4 fp16 stash copies each) + `nhalves`
        batched-chain units computing a, b from the stashed means."""
        S1 = pS.tile([P, 2 * FW], F16, tag="mImp", name="mImp")
        S2 = pS.tile([P, 2 * FW], F16, tag="cIpII", name="cIpII")

        def mI(s):
            return S1[:, s.start:s.stop]

        def mp(s):
            return S1[:, FW + s.start:FW + s.stop]

        def cIp(s):
            return S2[:, s.start:s.stop]

        def cII(s):
            return S2[:, FW + s.start:FW + s.stop]

        u = pS1.tile([P, FW], F16, tag="u", name="u")
        v = pS1.tile([P, FW], F16, tag="v", name="v")
        cov = pS1.tile([P, FW], F16, tag="cov", name="cov")
        den = pS1.tile([P, FW], F16, tag="den", name="den")
        t = pS1.tile([P, FW], F16, tag="t", name="t")
        a_f = pAB.tile([P, FW], F16, tag="af", name="af")
        b_f = pAB.tile([P, FW], F16, tag="bf", name="bf")
        st["a_f"], st["b_f"] = a_f, b_f
        for j in range(NB):
            # paired 2-bank PSUM tiles: qI|qp and qIp|qII
            q12 = pQ.tile([P, 2 * HW_], F32, tag="qIP")
            q34 = pQ.tile([P, 2 * HW_], F32, tag="qIpII")
            wpass_mm(st["yI"], q12[:, 0:HW_], j)
            wpass_mm(st["yp"], q12[:, HW_:2 * HW_], j)
            wpass_mm(st["yIp"], q34[:, 0:HW_], j)
            wpass_mm(st["yII"], q34[:, HW_:2 * HW_], j)
            sl = slice(j * HW_, (j + 1) * HW_)
            # free each q pair with one paired fp16 stash copy
            dst1 = S1[:].rearrange("p (s x) -> p s x", s=2)[:, :, sl]
            dst2 = S2[:].rearrange("p (s x) -> p s x", s=2)[:, :, sl]
            nc.scalar.copy(dst1, q12[:].rearrange("p (s x) -> p s x", s=2))
            nc.vector.tensor_copy(
                dst2, q34[:].rearrange("p (s x) -> p s x", s=2))
            # u, v per-j on Pool: off the a-chain critical path
            nc.gpsimd.tensor_tensor(u[:, sl], mI(sl), mp(sl), op=ALU.mult)
            nc.gpsimd.tensor_tensor(v[:, sl], mI(sl), mI(sl), op=ALU.mult)
            yield
        # batched chain: fp16 2-tensor ops in DVE 2x mode
        hw = FW // nhalves
        for h in range(nhalves):
            sl = slice(h * hw, (h + 1) * hw)
            nc.vector.tensor_tensor(cov[:, sl], cIp(sl), u[:, sl],
                                    op=ALU.subtract)                 # DVE 2x
            nc.vector.tensor_tensor(den[:, sl], cII(sl), v[:, sl],
                                    op=ALU.subtract)                 # DVE 2x
            nc.vector._custom_dve(
                _get_mul_recip_op(), out=a_f[:, sl], in0=den[:, sl],
                in1=cov[:, sl], s0=-0.23549792, s1=2.0017324, imm2=EPS)
            nc.vector.tensor_tensor(t[:, sl], mI(sl), a_f[:, sl],
                                    op=ALU.mult)                     # DVE 2x
            nc.vector.tensor_tensor(b_f[:, sl], mp(sl), t[:, sl],
                                    op=ALU.subtract)                 # DVE 2x
            yield

    def stageC_units(c, st):
        """6 units: V(a), V(b), then per-j W-passes + combine + out DMA.
        ra/rb reuse the stage-B q banks (pQ pool)."""
        a_f, b_f, I_f = st["a_f"], st["b_f"], st["I_f"]
        ya = pY.tile([P, FW], F16, tag="ya", name="ya")
        yb = pY.tile([P, FW], F16, tag="yb", name="yb")
        vpass(a_f, 0, ya, ["act", "act"])
        yield
        vpass(b_f, 0, yb, ["act", "act"])
        yield

        out_t = pOut.tile([P, FW], F32, tag="out", name="out")
        for j in range(NB):
            rab = pQ.tile([P, 2 * HW_], F32, tag="qIpII", name="rab")
            wpass_mm(ya, rab[:, 0:HW_], j)
            wpass_mm(yb, rab[:, HW_:2 * HW_], j)
            sl = slice(j * HW_, (j + 1) * HW_)
            f1 = pS.tile([P, HW_], F32, tag="f1", name="f1")
            nc.vector.scalar_tensor_tensor(
                f1[:], rab[:, 0:HW_], 1.0, I_f[:, sl],
                op0=ALU.mult, op1=ALU.mult)
            nc.vector.scalar_tensor_tensor(
                out_t[:, sl], rab[:, HW_:2 * HW_], 1.0, f1[:],
                op0=ALU.mult, op1=ALU.add)
            nc.sync.dma_start(
                out_d[c].rearrange("(hb hp) w -> hp hb w", hp=P)[:, j, :],
                out_t[:, sl])
            yield

    # -- fine-grained software pipeline over the 3 images ------------------
    sts = [{}, {}, {}]
    issue_dma(0, sts[0])
    A = [stageA_units(sts[c]) for c in range(CH)]
    B = [stageB_units(sts[0], 1), stageB_units(sts[1], 1),
         stageB_units(sts[2], 2)]
    C = [stageC_units(c, sts[c]) for c in range(CH)]

    def run(gen):
        next(gen)

    for _ in range(4):
        run(A[0])
    issue_dma(1, sts[1])
    for _ in range(4):
        run(B[0]); run(A[1])
    run(B[0])                 # B0 chain (batched)
    issue_dma(2, sts[2])
    for _ in range(4):
        run(B[1]); run(A[2])
    run(B[1])                 # B1 chain
    run(C[0]); run(B[2])      # C0.va   B2.j0
    run(C[0]); run(B[2])      # C0.vb   B2.j1
    run(C[0]); run(B[2])      # C0.j0   B2.j2
    run(C[0]); run(B[2])      # C0.j1   B2.j3
    run(C[0]); run(B[2])      # C0.j2   B2 chain half 0
    run(C[0]); run(B[2])      # C0.j3   B2 chain half 1
    for _ in range(6):
        run(C[1]); run(C[2])


_NC_CACHE = None
LAST_RESULT = None


def _get_model():
    global _NC_CACHE
    if _NC_CACHE is None:
        _NC_CACHE = build_model()
    return _NC_CACHE


def kernel(I, p):
    global LAST_RESULT
    I = np.asarray(I, dtype=np.float32)
    p = np.asarray(p, dtype=np.float32)
    B = I.shape[0]
    assert I.shape == (B, CH, HW_, HW_), I.shape
    nc = _get_model()
    consts = make_consts()
    I_16 = I.astype(np.float16)
    p_16 = p.astype(np.float16)
    Ip_16 = (I_16.astype(np.float32) * p_16.astype(np.float32)).astype(
        np.float16)
    II_16 = (I * I).astype(np.float16)
    Q = np.ascontiguousarray(
        np.stack([I_16, p_16, Ip_16, II_16], axis=2))  # [B, CH, NQ, H, W]
    in_maps = []
    for k in range(NCORES):
        m = {"I": np.ascontiguousarray(I[k]), "Qf16": Q[k]}
        m.update(consts)
        in_maps.append(m)
    kwargs = {}
    if os.environ.get("BASS_TRACE_DIR"):
        kwargs["tmpdir"] = os.environ["BASS_TRACE_DIR"]
    res = run_bass_kernel_spmd(nc, in_maps, core_ids=list(range(NCORES)), **kwargs)
    LAST_RESULT = res
    out = np.stack([res.results[k]["out"] for k in range(NCORES)], axis=0)
    return out.astype(np.float32)


if __name__ == "__main__":
    rng = np.random.default_rng(0)
    I = rng.random((8, CH, HW_, HW_), dtype=np.float32)
    p = rng.random((8, CH, HW_, HW_), dtype=np.float32)
    out = kernel(I, p)
    print("out", out.shape, out.dtype, float(out.mean()))


# revision 28
# speedup vs baseline: 1.0821x; 1.0821x over previous
"""Guided filter (r=40, eps=1e-3) on 8 Trainium2 NeuronCores.

Sharding: pure data-parallel over the batch dim (8 batches -> 8 cores).
Each core processes 3 channel-images of 512x512.

Algorithm per image:
  box2d(x) as two banded matmuls on the TensorEngine (image chunk is the
  stationary operand, the 0/1 band matrix the moving operand; contraction
  runs over the partition dim so each pass both box-filters one axis and
  transposes the layout).  Everything on-chip is fp16: the band matrices
  carry fp16(1/n) directly (rel err ~3e-4, so no residual correction is
  needed anywhere), the host pre-packs fp16(I), fp16(p), fp16(I*p),
  fp16(I*I) into one stacked DRAM tensor.
  Stage B: each W-pass PSUM bank is freed by one fp16 stash copy; the
  elementwise chain then runs batched at [128,2048] (images 0,1) or
  [128,1024] halves (image 2, to keep the pipeline tail short) using
  DVE 2x fp16 tensor_tensor ops.  ~36 PE warmup matmuls at t=0 bridge
  the initial DMA wait and keep the PE HAM clock-gate warm.
"""

import os
import sys
import numpy as np
import ml_dtypes
from contextlib import ExitStack

sys.path.insert(0, "/opt/trn_rl_repo")

import concourse.bass as bass
import concourse.tile as tile
from concourse import bacc, mybir
from concourse.bass_utils import run_bass_kernel_spmd

F32 = mybir.dt.float32
F16 = mybir.dt.float16
ALU = mybir.AluOpType

R = 40
EPS = 1e-3
HW_ = 512
NB = 4  # 128-row blocks per axis
CH = 3  # channels per batch
NQ = 4  # stacked fp16 quantities: I, p, I*p, I*I
P = 128
NCORES = 8
N_WARMUP_MM = 16


_MUL_RECIP_OP = None


def _get_mul_recip_op():
    """Register a fused custom-DVE op: out = Src1 * recip_approx(Src0+C2),
    BITWISE_NOT exponent-flip seed + one inline Newton step (~0.4% rel err,
    one DVE pass instead of reciprocal + tensor_mul)."""
    global _MUL_RECIP_OP
    if _MUL_RECIP_OP is not None:
        return _MUL_RECIP_OP
    import re
    import concourse.dve_ops as dops
    from concourse.dve_spec import AluOp, Bin, C0, C1, C2, Spec, Src0, Src1

    name = "MUL_RECIP_EPS_GF"
    _x = Src0 + C2
    _not_x = Bin(AluOp.BITWISE_NOT, _x, _x)
    _y0 = _not_x * C0

    def _ref(in0, in1, c0, c1, c2):
        x = in0 + c2
        not_x = (~x.view(np.int32)).view(np.float32)
        y0 = not_x * c0
        return in1 * (y0 * (c1 - x * y0))

    op = dops.DveOp(
        name, Spec(body=Src1 * (_y0 * (C1 - _x * _y0)), reference=_ref),
        subdim=False, uops_sha={})
    dops.OPS.append(op)
    dops.CUSTOM_DVE_SPECS[name] = op.spec
    dops._SUB_OPCODE_FOR_NAME[name] = max(dops._SUB_OPCODE_FOR_NAME.values()) + 1
    for ver in ("v3", "v4"):
        try:
            op.compile(ver)
        except ValueError as e:
            m = re.search(r'uops_sha\["%s"\]="([0-9a-f]+)"' % ver, str(e))
            if not m:
                raise
            op.uops_sha[ver] = m.group(1)
            dops._COMPILE_CACHE.pop((name, ver), None)
            op.compile(ver)
    _MUL_RECIP_OP = op
    return op


def _band_range(c):
    n0 = max(0, P * c - R)
    n1 = min(HW_, P * c + P + R)
    return n0, n1


_BAND_OFF = []
_BAND_W = []
_off = 0
for _c in range(NB):
    _n0, _n1 = _band_range(_c)
    _BAND_OFF.append(_off)
    _BAND_W.append(_n1 - _n0)
    _off += _n1 - _n0
BAND_TOT = _off  # 792


def make_consts():
    idx = np.arange(HW_)
    n1d = (np.minimum(idx + R, HW_ - 1) - np.maximum(idx - R, 0) + 1).astype(np.float64)
    inv_n = 1.0 / n1d

    mask = (np.abs(idx[:, None] - idx[None, :]) <= R)
    band = (mask * inv_n[None, :]).astype(np.float16)
    # [512k, 512n] -> [128 kp, NB, 512] then pack only the band cols
    band = band.reshape(NB, P, HW_).transpose(1, 0, 2)
    cols = []
    for c in range(NB):
        n0, n1 = _band_range(c)
        cols.append(band[:, c, n0:n1])
    return {"band": np.ascontiguousarray(np.concatenate(cols, axis=1))}


def build_model():
    nc = bacc.Bacc("TRN2", target_bir_lowering=False, debug=False,
                   num_devices=NCORES)
    I_d = nc.dram_tensor("I", [CH, HW_, HW_], F32, kind="ExternalInput").ap()
    Q_d = nc.dram_tensor("Qf16", [CH, NQ, HW_, HW_], F16,
                         kind="ExternalInput").ap()
    band_d = nc.dram_tensor("band", [P, BAND_TOT], F16, kind="ExternalInput").ap()
    out_d = nc.dram_tensor("out", [CH, HW_, HW_], F32, kind="ExternalOutput").ap()

    with tile.TileContext(nc) as tc:
        with ExitStack() as ctx:
            build_kernel(ctx, tc, I_d, Q_d, out_d, band_d)
    nc.compile()
    return nc


def build_kernel(ctx, tc, I_d, Q_d, out_d, band_d):
    nc = tc.nc
    FW = NB * HW_    # 2048 free cols per quantity-image
    QW = NQ * FW     # 8192 free cols for the 4 stacked quantities
    HF = FW // 2     # 1024

    pQin = ctx.enter_context(tc.tile_pool(name="qin", bufs=2))
    pIf = ctx.enter_context(tc.tile_pool(name="If", bufs=3))
    consts = ctx.enter_context(tc.tile_pool(name="consts", bufs=1))
    pY = ctx.enter_context(tc.tile_pool(name="ymid", bufs=2))
    pAB = ctx.enter_context(tc.tile_pool(name="ab", bufs=2))
    pOut = ctx.enter_context(tc.tile_pool(name="outp", bufs=2))
    pS = ctx.enter_context(tc.tile_pool(name="stash", bufs=2))
    pS1 = ctx.enter_context(tc.tile_pool(name="chain", bufs=1))
    pV = ctx.enter_context(tc.tile_pool(name="psv", bufs=2, space="PSUM"))
    pQ = ctx.enter_context(tc.tile_pool(name="psq", bufs=1, space="PSUM"))

    # -- PE warmup: ~36 matmuls on a memset tile, no DMA dependencies. ----
    # Bridges the initial input-DMA wait and releases the HAM clock gate
    # before real matmuls arrive.
    wsrc = consts.tile([P, HW_], F16, tag="wsrc", name="wsrc")
    nc.gpsimd.memset(wsrc[:], 0.0)
    for k in range(N_WARMUP_MM):
        ps = pV.tile([P, 2 * HW_], F32, tag="ps")
        for i2 in range(2):
            nc.tensor.matmul(ps[:, i2 * HW_:i2 * HW_ + 256],
                             lhsT=wsrc[:, 0:P], rhs=wsrc[:, 0:256],
                             start=True, stop=True)

    def issue_dma(c, st):
        """Prefetch image c's inputs on the sync HWDGE queue: the guide
        image first (it gates the first V-pass), then p/Ip/II, then the
        f32 guide (only needed at stage C)."""
        Q = pQin.tile([P, QW], F16, tag="Qf16", name="Qf16")
        I_f = pIf.tile([P, FW], F32, tag="If", name="If")
        nc.sync.dma_start(
            Q[:, 0:2 * FW].rearrange("p (q hb w) -> p q hb w", q=2, w=HW_),
            Q_d[c, 0:2].rearrange("q (hb hp) w -> hp q hb w", hp=P))
        nc.sync.dma_start(
            Q[:, 2 * FW:QW].rearrange("p (q hb w) -> p q hb w", q=2, w=HW_),
            Q_d[c, 2:4].rearrange("q (hb hp) w -> hp q hb w", hp=P))
        nc.sync.dma_start(
            I_f[:].rearrange("p (hb w) -> p hb w", w=HW_),
            I_d[c].rearrange("(hb hp) w -> hp hb w", hp=P))
        st["Q"], st["I_f"] = Q, I_f

    band = consts.tile_from(band_d)

    def vpass(src, src_off, dst, engines):
        """One banded V-pass: src [h|w] fp16 -> dst [w|h] fp16 (box over
        partition axis + transpose). 16 MMs into two 2-bank PSUM tiles,
        each drained by one [128,1024] copy (engine per-half)."""
        for half in range(2):
            ps = pV.tile([P, 2 * HW_], F32, tag="ps")
            for i2 in range(2):
                i = 2 * half + i2
                for j in range(NB):
                    n0, n1 = _band_range(j)
                    o = src_off + j * HW_ + i * P
                    nc.tensor.matmul(
                        ps[:, i2 * HW_ + n0: i2 * HW_ + n1],
                        lhsT=src[:, o: o + P],
                        rhs=band[:, _BAND_OFF[j]: _BAND_OFF[j] + _BAND_W[j]],
                        start=(j == 0), stop=(j == NB - 1))
            d = dst[:, 2 * half * HW_: 2 * (half + 1) * HW_]
            if engines[half] == "dve":
                nc.vector.tensor_copy(d, ps[:])
            else:
                nc.scalar.copy(d, ps[:])

    def wpass_mm(src, q_tile, j):
        """W-direction banded MMs for output h-chunk j into q_tile."""
        for i in range(NB):
            m0, m1 = _band_range(i)
            nc.tensor.matmul(
                q_tile[:, m0:m1],
                lhsT=src[:, i * HW_ + j * P: i * HW_ + j * P + P],
                rhs=band[:, _BAND_OFF[i]: _BAND_OFF[i] + _BAND_W[i]],
                start=(i == 0), stop=(i == NB - 1))

    def stageA_units(st):
        """4 units: V-passes for I, p, Ip, II from the stacked input."""
        Q = st["Q"]
        for q, ytag in enumerate(("yI", "yp", "yIp", "yII")):
            y = pY.tile([P, FW], F16, tag=ytag, name=ytag)
            st[ytag] = y
            vpass(Q, q * FW, y, ["act", "act"])
            yield

    def stageB_units(st, nhalves):
        """4 j-units (W-passes + 4 fp16 stash copies each) + `nhalves`
        batched-chain units computing a, b from the stashed means."""
        S1 = pS.tile([P, 2 * FW], F16, tag="mImp", name="mImp")
        S2 = pS.tile([P, 2 * FW], F16, tag="cIpII", name="cIpII")

        def mI(s):
            return S1[:, s.start:s.stop]

        def mp(s):
            return S1[:, FW + s.start:FW + s.stop]

        def cIp(s):
            return S2[:, s.start:s.stop]

        def cII(s):
            return S2[:, FW + s.start:FW + s.stop]

        u = pS1.tile([P, FW], F16, tag="u", name="u")
        v = pS1.tile([P, FW], F16, tag="v", name="v")
        cov = pS1.tile([P, FW], F16, tag="cov", name="cov")
        den = pS1.tile([P, FW], F16, tag="den", name="den")
        t = pS1.tile([P, FW], F16, tag="t", name="t")
        a_f = pAB.tile([P, FW], F16, tag="af", name="af")
        b_f = pAB.tile([P, FW], F16, tag="bf", name="bf")
        st["a_f"], st["b_f"] = a_f, b_f
        for j in range(NB):
            # paired 2-bank PSUM tiles: qI|qp and qIp|qII
            q12 = pQ.tile([P, 2 * HW_], F32, tag="qIP")
            q34 = pQ.tile([P, 2 * HW_], F32, tag="qIpII")
            wpass_mm(st["yI"], q12[:, 0:HW_], j)
            wpass_mm(st["yp"], q12[:, HW_:2 * HW_], j)
            wpass_mm(st["yIp"], q34[:, 0:HW_], j)
            wpass_mm(st["yII"], q34[:, HW_:2 * HW_], j)
            sl = slice(j * HW_, (j + 1) * HW_)
            # free each q pair with one paired fp16 stash copy
            dst1 = S1[:].rearrange("p (s x) -> p s x", s=2)[:, :, sl]
            dst2 = S2[:].rearrange("p (s x) -> p s x", s=2)[:, :, sl]
            nc.scalar.copy(dst1, q12[:].rearrange("p (s x) -> p s x", s=2))
            nc.vector.tensor_copy(
                dst2, q34[:].rearrange("p (s x) -> p s x", s=2))
            # u, v per-j on Pool: off the a-chain critical path
            nc.gpsimd.tensor_tensor(u[:, sl], mI(sl), mp(sl), op=ALU.mult)
            nc.gpsimd.tensor_tensor(v[:, sl], mI(sl), mI(sl), op=ALU.mult)
            yield
        # batched chain: fp16 2-tensor ops in DVE 2x mode
        hw = FW // nhalves
        for h in range(nhalves):
            sl = slice(h * hw, (h + 1) * hw)
            nc.vector.tensor_tensor(cov[:, sl], cIp(sl), u[:, sl],
                                    op=ALU.subtract)                 # DVE 2x
            nc.vector.tensor_tensor(den[:, sl], cII(sl), v[:, sl],
                                    op=ALU.subtract)                 # DVE 2x
            nc.vector._custom_dve(
                _get_mul_recip_op(), out=a_f[:, sl], in0=den[:, sl],
                in1=cov[:, sl], s0=-0.23549792, s1=2.0017324, imm2=EPS)
            nc.vector.tensor_tensor(t[:, sl], mI(sl), a_f[:, sl],
                                    op=ALU.mult)                     # DVE 2x
            nc.vector.tensor_tensor(b_f[:, sl], mp(sl), t[:, sl],
                                    op=ALU.subtract)                 # DVE 2x
            yield

    def stageC_units(c, st):
        """6 units: V(a), V(b), then per-j W-passes + combine + out DMA.
        ra/rb reuse the stage-B q banks (pQ pool)."""
        a_f, b_f, I_f = st["a_f"], st["b_f"], st["I_f"]
        ya = pY.tile([P, FW], F16, tag="ya", name="ya")
        yb = pY.tile([P, FW], F16, tag="yb", name="yb")
        vpass(a_f, 0, ya, ["act", "act"])
        yield
        vpass(b_f, 0, yb, ["act", "act"])
        yield

        out_t = pOut.tile([P, FW], F32, tag="out", name="out")
        rtag = "qIP" if c % 2 == 0 else "qIpII"
        for j in range(NB):
            rab = pQ.tile([P, 2 * HW_], F32, tag=rtag, name="rab")
            wpass_mm(ya, rab[:, 0:HW_], j)
            wpass_mm(yb, rab[:, HW_:2 * HW_], j)
            sl = slice(j * HW_, (j + 1) * HW_)
            f1 = pS.tile([P, HW_], F32, tag="f1", name="f1")
            nc.vector.scalar_tensor_tensor(
                f1[:], rab[:, 0:HW_], 1.0, I_f[:, sl],
                op0=ALU.mult, op1=ALU.mult)
            nc.vector.scalar_tensor_tensor(
                out_t[:, sl], rab[:, HW_:2 * HW_], 1.0, f1[:],
                op0=ALU.mult, op1=ALU.add)
            nc.sync.dma_start(
                out_d[c].rearrange("(hb hp) w -> hp hb w", hp=P)[:, j, :],
                out_t[:, sl])
            yield

    # -- fine-grained software pipeline over the 3 images ------------------
    sts = [{}, {}, {}]
    issue_dma(0, sts[0])
    A = [stageA_units(sts[c]) for c in range(CH)]
    B = [stageB_units(sts[0], 1), stageB_units(sts[1], 1),
         stageB_units(sts[2], 2)]
    C = [stageC_units(c, sts[c]) for c in range(CH)]

    def run(gen):
        next(gen)

    for _ in range(4):
        run(A[0])
    issue_dma(1, sts[1])
    for _ in range(4):
        run(B[0]); run(A[1])
    run(B[0])                 # B0 chain (batched)
    issue_dma(2, sts[2])
    for _ in range(4):
        run(B[1]); run(A[2])
    run(B[1])                 # B1 chain
    run(C[0]); run(B[2])      # C0.va   B2.j0
    run(C[0]); run(B[2])      # C0.vb   B2.j1
    run(C[0]); run(B[2])      # C0.j0   B2.j2
    run(C[0]); run(B[2])      # C0.j1   B2.j3
    run(C[0]); run(B[2])      # C0.j2   B2 chain half 0
    run(C[0]); run(B[2])      # C0.j3   B2 chain half 1
    for _ in range(6):
        run(C[1]); run(C[2])


_NC_CACHE = None
LAST_RESULT = None


def _get_model():
    global _NC_CACHE
    if _NC_CACHE is None:
        _NC_CACHE = build_model()
    return _NC_CACHE


def kernel(I, p):
    global LAST_RESULT
    I = np.asarray(I, dtype=np.float32)
    p = np.asarray(p, dtype=np.float32)
    B = I.shape[0]
    assert I.shape == (B, CH, HW_, HW_), I.shape
    nc = _get_model()
    consts = make_consts()
    I_16 = I.astype(np.float16)
    p_16 = p.astype(np.float16)
    Ip_16 = (I_16.astype(np.float32) * p_16.astype(np.float32)).astype(
        np.float16)
    II_16 = (I * I).astype(np.float16)
    Q = np.ascontiguousarray(
        np.stack([I_16, p_16, Ip_16, II_16], axis=2))  # [B, CH, NQ, H, W]
    in_maps = []
    for k in range(NCORES):
        m = {"I": np.ascontiguousarray(I[k]), "Qf16": Q[k]}
        m.update(consts)
        in_maps.append(m)
    kwargs = {}
    if os.environ.get("BASS_TRACE_DIR"):
        kwargs["tmpdir"] = os.environ["BASS_TRACE_DIR"]
    res = run_bass_kernel_spmd(nc, in_maps, core_ids=list(range(NCORES)), **kwargs)
    LAST_RESULT = res
    out = np.stack([res.results[k]["out"] for k in range(NCORES)], axis=0)
    return out.astype(np.float32)


if __name__ == "__main__":
    rng = np.random.default_rng(0)
    I = rng.random((8, CH, HW_, HW_), dtype=np.float32)
    p = rng.random((8, CH, HW_, HW_), dtype=np.float32)
    out = kernel(I, p)
    print("out", out.shape, out.dtype, float(out.mean()))


# revision 30
# speedup vs baseline: 1.1370x; 1.0508x over previous
"""Guided filter (r=40, eps=1e-3) on 8 Trainium2 NeuronCores.

Sharding: pure data-parallel over the batch dim (8 batches -> 8 cores).
Each core processes 3 channel-images of 512x512.

Algorithm per image:
  box2d(x) as two banded matmuls on the TensorEngine (image chunk is the
  stationary operand, the 0/1 band matrix the moving operand; contraction
  runs over the partition dim so each pass both box-filters one axis and
  transposes the layout).  Everything on-chip is fp16: the band matrices
  carry fp16(1/n) directly (rel err ~3e-4, so no residual correction is
  needed anywhere), the host pre-packs fp16(I), fp16(p), fp16(I*p),
  fp16(I*I) into one stacked DRAM tensor.
  Stage B: each W-pass PSUM bank is freed by one fp16 stash copy; the
  elementwise chain then runs batched at [128,2048] (images 0,1) or
  [128,1024] halves (image 2, to keep the pipeline tail short) using
  DVE 2x fp16 tensor_tensor ops.  ~36 PE warmup matmuls at t=0 bridge
  the initial DMA wait and keep the PE HAM clock-gate warm.
"""

import os
import sys
import numpy as np
import ml_dtypes
from contextlib import ExitStack

sys.path.insert(0, "/opt/trn_rl_repo")

import concourse.bass as bass
import concourse.tile as tile
from concourse import bacc, mybir
from concourse.bass_utils import run_bass_kernel_spmd

F32 = mybir.dt.float32
F16 = mybir.dt.float16
ALU = mybir.AluOpType

R = 40
EPS = 1e-3
HW_ = 512
NB = 4  # 128-row blocks per axis
CH = 3  # channels per batch
NQ = 4  # stacked fp16 quantities: I, p, I*p, I*I
P = 128
NCORES = 8
N_WARMUP_MM = 16


_MUL_RECIP_OP = None


def _get_mul_recip_op():
    """Register a fused custom-DVE op: out = Src1 * recip_approx(Src0+C2),
    BITWISE_NOT exponent-flip seed + one inline Newton step (~0.4% rel err,
    one DVE pass instead of reciprocal + tensor_mul)."""
    global _MUL_RECIP_OP
    if _MUL_RECIP_OP is not None:
        return _MUL_RECIP_OP
    import re
    import concourse.dve_ops as dops
    from concourse.dve_spec import AluOp, Bin, C0, C1, C2, Spec, Src0, Src1

    name = "MUL_RECIP_EPS_GF"
    _x = Src0 + C2
    _not_x = Bin(AluOp.BITWISE_NOT, _x, _x)
    _y0 = _not_x * C0

    def _ref(in0, in1, c0, c1, c2):
        x = in0 + c2
        not_x = (~x.view(np.int32)).view(np.float32)
        y0 = not_x * c0
        return in1 * (y0 * (c1 - x * y0))

    op = dops.DveOp(
        name, Spec(body=Src1 * (_y0 * (C1 - _x * _y0)), reference=_ref),
        subdim=False, uops_sha={})
    dops.OPS.append(op)
    dops.CUSTOM_DVE_SPECS[name] = op.spec
    dops._SUB_OPCODE_FOR_NAME[name] = max(dops._SUB_OPCODE_FOR_NAME.values()) + 1
    for ver in ("v3", "v4"):
        try:
            op.compile(ver)
        except ValueError as e:
            m = re.search(r'uops_sha\["%s"\]="([0-9a-f]+)"' % ver, str(e))
            if not m:
                raise
            op.uops_sha[ver] = m.group(1)
            dops._COMPILE_CACHE.pop((name, ver), None)
            op.compile(ver)
    _MUL_RECIP_OP = op
    return op


def _band_range(c):
    n0 = max(0, P * c - R)
    n1 = min(HW_, P * c + P + R)
    return n0, n1


_BAND_OFF = []
_BAND_W = []
_off = 0
for _c in range(NB):
    _n0, _n1 = _band_range(_c)
    _BAND_OFF.append(_off)
    _BAND_W.append(_n1 - _n0)
    _off += _n1 - _n0
BAND_TOT = _off  # 792


def make_consts():
    idx = np.arange(HW_)
    n1d = (np.minimum(idx + R, HW_ - 1) - np.maximum(idx - R, 0) + 1).astype(np.float64)
    inv_n = 1.0 / n1d

    mask = (np.abs(idx[:, None] - idx[None, :]) <= R)
    band = (mask * inv_n[None, :]).astype(np.float16)
    # [512k, 512n] -> [128 kp, NB, 512] then pack only the band cols
    band = band.reshape(NB, P, HW_).transpose(1, 0, 2)
    cols = []
    for c in range(NB):
        n0, n1 = _band_range(c)
        cols.append(band[:, c, n0:n1])
    return {"band": np.ascontiguousarray(np.concatenate(cols, axis=1))}


def build_model():
    nc = bacc.Bacc("TRN2", target_bir_lowering=False, debug=False,
                   num_devices=NCORES)
    I_d = nc.dram_tensor("I", [CH, HW_, HW_], F32, kind="ExternalInput").ap()
    Q_d = nc.dram_tensor("Qf16", [CH, NQ, HW_, HW_], F16,
                         kind="ExternalInput").ap()
    band_d = nc.dram_tensor("band", [P, BAND_TOT], F16, kind="ExternalInput").ap()
    out_d = nc.dram_tensor("out", [CH, HW_, HW_], F32, kind="ExternalOutput").ap()

    with tile.TileContext(nc) as tc:
        with ExitStack() as ctx:
            build_kernel(ctx, tc, I_d, Q_d, out_d, band_d)
    nc.compile()
    return nc


def build_kernel(ctx, tc, I_d, Q_d, out_d, band_d):
    nc = tc.nc
    FW = NB * HW_    # 2048 free cols per quantity-image
    QW = NQ * FW     # 8192 free cols for the 4 stacked quantities
    HF = FW // 2     # 1024

    pQin = ctx.enter_context(tc.tile_pool(name="qin", bufs=2))
    pIf = ctx.enter_context(tc.tile_pool(name="If", bufs=3))
    consts = ctx.enter_context(tc.tile_pool(name="consts", bufs=1))
    pY = ctx.enter_context(tc.tile_pool(name="ymid", bufs=2))
    pAB = ctx.enter_context(tc.tile_pool(name="ab", bufs=2))
    pOut = ctx.enter_context(tc.tile_pool(name="outp", bufs=2))
    pS = ctx.enter_context(tc.tile_pool(name="stash", bufs=2))
    pS1 = ctx.enter_context(tc.tile_pool(name="chain", bufs=1))
    pV = ctx.enter_context(tc.tile_pool(name="psv", bufs=2, space="PSUM"))
    pQ = ctx.enter_context(tc.tile_pool(name="psq", bufs=1, space="PSUM"))

    # -- PE warmup: ~36 matmuls on a memset tile, no DMA dependencies. ----
    # Bridges the initial input-DMA wait and releases the HAM clock gate
    # before real matmuls arrive.
    wsrc = consts.tile([P, HW_], F16, tag="wsrc", name="wsrc")
    nc.gpsimd.memset(wsrc[:], 0.0)
    for k in range(N_WARMUP_MM):
        ps = pV.tile([P, 2 * HW_], F32, tag="ps")
        for i2 in range(2):
            nc.tensor.matmul(ps[:, i2 * HW_:i2 * HW_ + 256],
                             lhsT=wsrc[:, 0:P], rhs=wsrc[:, 0:256],
                             start=True, stop=True)

    def issue_dma(c, st):
        """Prefetch image c's inputs on the sync HWDGE queue: the guide
        image first (it gates the first V-pass), then p/Ip/II, then the
        f32 guide (only needed at stage C)."""
        Q = pQin.tile([P, QW], F16, tag="Qf16", name="Qf16")
        I_f = pIf.tile([P, FW], F32, tag="If", name="If")
        nc.sync.dma_start(
            Q[:, 0:2 * FW].rearrange("p (q hb w) -> p q hb w", q=2, w=HW_),
            Q_d[c, 0:2].rearrange("q (hb hp) w -> hp q hb w", hp=P))
        nc.sync.dma_start(
            Q[:, 2 * FW:QW].rearrange("p (q hb w) -> p q hb w", q=2, w=HW_),
            Q_d[c, 2:4].rearrange("q (hb hp) w -> hp q hb w", hp=P))
        nc.sync.dma_start(
            I_f[:].rearrange("p (hb w) -> p hb w", w=HW_),
            I_d[c].rearrange("(hb hp) w -> hp hb w", hp=P))
        st["Q"], st["I_f"] = Q, I_f

    band = consts.tile_from(band_d)

    def vpass(src, src_off, dst, engines):
        """One banded V-pass: src [h|w] fp16 -> dst [w|h] fp16 (box over
        partition axis + transpose). 16 MMs into two 2-bank PSUM tiles,
        each drained by one [128,1024] copy (engine per-half)."""
        for half in range(2):
            ps = pV.tile([P, 2 * HW_], F32, tag="ps")
            for i2 in range(2):
                i = 2 * half + i2
                for j in range(NB):
                    n0, n1 = _band_range(j)
                    o = src_off + j * HW_ + i * P
                    nc.tensor.matmul(
                        ps[:, i2 * HW_ + n0: i2 * HW_ + n1],
                        lhsT=src[:, o: o + P],
                        rhs=band[:, _BAND_OFF[j]: _BAND_OFF[j] + _BAND_W[j]],
                        start=(j == 0), stop=(j == NB - 1))
            d = dst[:, 2 * half * HW_: 2 * (half + 1) * HW_]
            if engines[half] == "dve":
                nc.vector.tensor_copy(d, ps[:])
            else:
                nc.scalar.copy(d, ps[:])

    def wpass_mm(src, q_tile, j):
        """W-direction banded MMs for output h-chunk j into q_tile."""
        for i in range(NB):
            m0, m1 = _band_range(i)
            nc.tensor.matmul(
                q_tile[:, m0:m1],
                lhsT=src[:, i * HW_ + j * P: i * HW_ + j * P + P],
                rhs=band[:, _BAND_OFF[i]: _BAND_OFF[i] + _BAND_W[i]],
                start=(i == 0), stop=(i == NB - 1))

    def stageA_units(st):
        """4 units: V-passes for I, p, Ip, II from the stacked input."""
        Q = st["Q"]
        for q, ytag in enumerate(("yI", "yp", "yIp", "yII")):
            y = pY.tile([P, FW], F16, tag=ytag, name=ytag)
            st[ytag] = y
            vpass(Q, q * FW, y, ["act", "act"])
            yield

    def stageB_units(st, nhalves):
        """4 j-units (W-passes + 4 fp16 stash copies each) + `nhalves`
        batched-chain units computing a, b from the stashed means."""
        S1 = pS.tile([P, 2 * FW], F16, tag="mImp", name="mImp")
        S2 = pS.tile([P, 2 * FW], F16, tag="cIpII", name="cIpII")

        def mI(s):
            return S1[:, s.start:s.stop]

        def mp(s):
            return S1[:, FW + s.start:FW + s.stop]

        def cIp(s):
            return S2[:, s.start:s.stop]

        def cII(s):
            return S2[:, FW + s.start:FW + s.stop]

        u = pS1.tile([P, FW], F16, tag="u", name="u")
        v = pS1.tile([P, FW], F16, tag="v", name="v")
        cov = pS1.tile([P, FW], F16, tag="cov", name="cov")
        den = pS1.tile([P, FW], F16, tag="den", name="den")
        t = pS1.tile([P, FW], F16, tag="t", name="t")
        a_f = pAB.tile([P, FW], F16, tag="af", name="af")
        b_f = pAB.tile([P, FW], F16, tag="bf", name="bf")
        st["a_f"], st["b_f"] = a_f, b_f
        for j in range(NB):
            qI = pQ.tile([P, HW_], F32, tag="qI")
            qp = pQ.tile([P, HW_], F32, tag="qp")
            qIp = pQ.tile([P, HW_], F32, tag="qIp")
            qII = pQ.tile([P, HW_], F32, tag="qII")
            wpass_mm(st["yI"], qI, j)
            wpass_mm(st["yp"], qp, j)
            wpass_mm(st["yIp"], qIp, j)
            wpass_mm(st["yII"], qII, j)
            sl = slice(j * HW_, (j + 1) * HW_)
            # free each q bank with one fp16 stash copy (ACT/DVE only)
            nc.scalar.copy(mI(sl), qI[:])
            nc.scalar.copy(mp(sl), qp[:])
            nc.vector.tensor_copy(cIp(sl), qIp[:])
            nc.vector.tensor_copy(cII(sl), qII[:])
            # u, v per-j on Pool: off the a-chain critical path
            nc.gpsimd.tensor_tensor(u[:, sl], mI(sl), mp(sl), op=ALU.mult)
            nc.gpsimd.tensor_tensor(v[:, sl], mI(sl), mI(sl), op=ALU.mult)
            yield
        # batched chain: fp16 2-tensor ops in DVE 2x mode
        hw = FW // nhalves
        for h in range(nhalves):
            sl = slice(h * hw, (h + 1) * hw)
            nc.vector.tensor_tensor(cov[:, sl], cIp(sl), u[:, sl],
                                    op=ALU.subtract)                 # DVE 2x
            nc.vector.tensor_tensor(den[:, sl], cII(sl), v[:, sl],
                                    op=ALU.subtract)                 # DVE 2x
            nc.vector._custom_dve(
                _get_mul_recip_op(), out=a_f[:, sl], in0=den[:, sl],
                in1=cov[:, sl], s0=-0.23549792, s1=2.0017324, imm2=EPS)
            nc.vector.tensor_tensor(t[:, sl], mI(sl), a_f[:, sl],
                                    op=ALU.mult)                     # DVE 2x
            nc.vector.tensor_tensor(b_f[:, sl], mp(sl), t[:, sl],
                                    op=ALU.subtract)                 # DVE 2x
            yield

    def stageC_units(c, st):
        """6 units: V(a), V(b), then per-j W-passes + combine + out DMA.
        ra/rb reuse the stage-B q banks (pQ pool)."""
        a_f, b_f, I_f = st["a_f"], st["b_f"], st["I_f"]
        ya = pY.tile([P, FW], F16, tag="ya", name="ya")
        yb = pY.tile([P, FW], F16, tag="yb", name="yb")
        vpass(a_f, 0, ya, ["act", "act"])
        yield
        vpass(b_f, 0, yb, ["act", "act"])
        yield

        out_t = pOut.tile([P, FW], F32, tag="out", name="out")
        rtags = ("qI", "qp") if c % 2 == 0 else ("qIp", "qII")
        for j in range(NB):
            ra = pQ.tile([P, HW_], F32, tag=rtags[0], name="ra")
            rb = pQ.tile([P, HW_], F32, tag=rtags[1], name="rb")
            wpass_mm(ya, ra, j)
            wpass_mm(yb, rb, j)
            sl = slice(j * HW_, (j + 1) * HW_)
            f1 = pS.tile([P, HW_], F32, tag="f1", name="f1")
            nc.vector.scalar_tensor_tensor(
                f1[:], ra[:], 1.0, I_f[:, sl], op0=ALU.mult, op1=ALU.mult)
            nc.vector.scalar_tensor_tensor(
                out_t[:, sl], rb[:], 1.0, f1[:], op0=ALU.mult, op1=ALU.add)
            nc.sync.dma_start(
                out_d[c].rearrange("(hb hp) w -> hp hb w", hp=P)[:, j, :],
                out_t[:, sl])
            yield

    # -- fine-grained software pipeline over the 3 images ------------------
    sts = [{}, {}, {}]
    issue_dma(0, sts[0])
    A = [stageA_units(sts[c]) for c in range(CH)]
    B = [stageB_units(sts[0], 1), stageB_units(sts[1], 1),
         stageB_units(sts[2], 2)]
    C = [stageC_units(c, sts[c]) for c in range(CH)]

    def run(gen):
        next(gen)

    for _ in range(4):
        run(A[0])
    issue_dma(1, sts[1])
    for _ in range(4):
        run(B[0]); run(A[1])
    run(B[0])                 # B0 chain (batched)
    issue_dma(2, sts[2])
    for _ in range(4):
        run(B[1]); run(A[2])
    run(B[1])                 # B1 chain
    run(C[0]); run(B[2])      # C0.va   B2.j0
    run(C[0]); run(B[2])      # C0.vb   B2.j1
    run(C[0]); run(B[2])      # C0.j0   B2.j2
    run(C[0]); run(B[2])      # C0.j1   B2.j3
    run(C[0]); run(B[2])      # C0.j2   B2 chain half 0
    run(C[0]); run(B[2])      # C0.j3   B2 chain half 1
    for _ in range(6):
        run(C[1]); run(C[2])


_NC_CACHE = None
LAST_RESULT = None


def _get_model():
    global _NC_CACHE
    if _NC_CACHE is None:
        _NC_CACHE = build_model()
    return _NC_CACHE


def kernel(I, p):
    global LAST_RESULT
    I = np.asarray(I, dtype=np.float32)
    p = np.asarray(p, dtype=np.float32)
    B = I.shape[0]
    assert I.shape == (B, CH, HW_, HW_), I.shape
    nc = _get_model()
    consts = make_consts()
    I_16 = I.astype(np.float16)
    p_16 = p.astype(np.float16)
    Ip_16 = (I_16.astype(np.float32) * p_16.astype(np.float32)).astype(
        np.float16)
    II_16 = (I * I).astype(np.float16)
    Q = np.ascontiguousarray(
        np.stack([I_16, p_16, Ip_16, II_16], axis=2))  # [B, CH, NQ, H, W]
    in_maps = []
    for k in range(NCORES):
        m = {"I": np.ascontiguousarray(I[k]), "Qf16": Q[k]}
        m.update(consts)
        in_maps.append(m)
    kwargs = {}
    if os.environ.get("BASS_TRACE_DIR"):
        kwargs["tmpdir"] = os.environ["BASS_TRACE_DIR"]
    res = run_bass_kernel_spmd(nc, in_maps, core_ids=list(range(NCORES)), **kwargs)
    LAST_RESULT = res
    out = np.stack([res.results[k]["out"] for k in range(NCORES)], axis=0)
    return out.astype(np.float32)


if __name__ == "__main__":
    rng = np.random.default_rng(0)
    I = rng.random((8, CH, HW_, HW_), dtype=np.float32)
    p = rng.random((8, CH, HW_, HW_), dtype=np.float32)
    out = kernel(I, p)
    print("out", out.shape, out.dtype, float(out.mean()))


# revision 34
# speedup vs baseline: 1.1461x; 1.0080x over previous
"""Guided filter (r=40, eps=1e-3) on 8 Trainium2 NeuronCores.

Sharding: pure data-parallel over the batch dim (8 batches -> 8 cores).
Each core processes 3 channel-images of 512x512.

Algorithm per image:
  box2d(x) as two banded matmuls on the TensorEngine (image chunk is the
  stationary operand, the 0/1 band matrix the moving operand; contraction
  runs over the partition dim so each pass both box-filters one axis and
  transposes the layout).  Everything on-chip is fp16: the band matrices
  carry fp16(1/n) directly (rel err ~3e-4, so no residual correction is
  needed anywhere), the host pre-packs fp16(I), fp16(p), fp16(I*p),
  fp16(I*I) into one stacked DRAM tensor.
  Stage B: each W-pass PSUM bank is freed by one fp16 stash copy; the
  elementwise chain then runs batched at [128,2048] (images 0,1) or
  [128,1024] halves (image 2, to keep the pipeline tail short) using
  DVE 2x fp16 tensor_tensor ops.  ~36 PE warmup matmuls at t=0 bridge
  the initial DMA wait and keep the PE HAM clock-gate warm.
"""

import os
import sys
import numpy as np
import ml_dtypes
from contextlib import ExitStack

sys.path.insert(0, "/opt/trn_rl_repo")

import concourse.bass as bass
import concourse.tile as tile
from concourse import bacc, mybir
from concourse.bass_utils import run_bass_kernel_spmd

F32 = mybir.dt.float32
F16 = mybir.dt.float16
ALU = mybir.AluOpType

R = 40
EPS = 1e-3
HW_ = 512
NB = 4  # 128-row blocks per axis
CH = 3  # channels per batch
NQ = 4  # stacked fp16 quantities: I, p, I*p, I*I
P = 128
NCORES = 8
N_WARMUP_MM = 16


_MUL_RECIP_OP = None


def _get_mul_recip_op():
    """Register a fused custom-DVE op: out = Src1 * recip_approx(Src0+C2),
    BITWISE_NOT exponent-flip seed + one inline Newton step (~0.4% rel err,
    one DVE pass instead of reciprocal + tensor_mul)."""
    global _MUL_RECIP_OP
    if _MUL_RECIP_OP is not None:
        return _MUL_RECIP_OP
    import re
    import concourse.dve_ops as dops
    from concourse.dve_spec import AluOp, Bin, C0, C1, C2, Spec, Src0, Src1

    name = "MUL_RECIP_EPS_GF"
    _x = Src0 + C2
    _not_x = Bin(AluOp.BITWISE_NOT, _x, _x)
    _y0 = _not_x * C0

    def _ref(in0, in1, c0, c1, c2):
        x = in0 + c2
        not_x = (~x.view(np.int32)).view(np.float32)
        y0 = not_x * c0
        return in1 * (y0 * (c1 - x * y0))

    op = dops.DveOp(
        name, Spec(body=Src1 * (_y0 * (C1 - _x * _y0)), reference=_ref),
        subdim=False, uops_sha={})
    dops.OPS.append(op)
    dops.CUSTOM_DVE_SPECS[name] = op.spec
    dops._SUB_OPCODE_FOR_NAME[name] = max(dops._SUB_OPCODE_FOR_NAME.values()) + 1
    for ver in ("v3", "v4"):
        try:
            op.compile(ver)
        except ValueError as e:
            m = re.search(r'uops_sha\["%s"\]="([0-9a-f]+)"' % ver, str(e))
            if not m:
                raise
            op.uops_sha[ver] = m.group(1)
            dops._COMPILE_CACHE.pop((name, ver), None)
            op.compile(ver)
    _MUL_RECIP_OP = op
    return op


def _band_range(c):
    n0 = max(0, P * c - R)
    n1 = min(HW_, P * c + P + R)
    return n0, n1


_BAND_OFF = []
_BAND_W = []
_off = 0
for _c in range(NB):
    _n0, _n1 = _band_range(_c)
    _BAND_OFF.append(_off)
    _BAND_W.append(_n1 - _n0)
    _off += _n1 - _n0
BAND_TOT = _off  # 792


def make_consts():
    idx = np.arange(HW_)
    n1d = (np.minimum(idx + R, HW_ - 1) - np.maximum(idx - R, 0) + 1).astype(np.float64)
    inv_n = 1.0 / n1d

    mask = (np.abs(idx[:, None] - idx[None, :]) <= R)
    band = (mask * inv_n[None, :]).astype(np.float16)
    # [512k, 512n] -> [128 kp, NB, 512] then pack only the band cols
    band = band.reshape(NB, P, HW_).transpose(1, 0, 2)
    cols = []
    for c in range(NB):
        n0, n1 = _band_range(c)
        cols.append(band[:, c, n0:n1])
    return {"band": np.ascontiguousarray(np.concatenate(cols, axis=1))}


def build_model():
    nc = bacc.Bacc("TRN2", target_bir_lowering=False, debug=False,
                   num_devices=NCORES)
    I_d = nc.dram_tensor("I", [CH, HW_, HW_], F32, kind="ExternalInput").ap()
    Q_d = nc.dram_tensor("Qf16", [CH, NQ, HW_, HW_], F16,
                         kind="ExternalInput").ap()
    band_d = nc.dram_tensor("band", [P, BAND_TOT], F16, kind="ExternalInput").ap()
    out_d = nc.dram_tensor("out", [CH, HW_, HW_], F32, kind="ExternalOutput").ap()

    with tile.TileContext(nc) as tc:
        with ExitStack() as ctx:
            build_kernel(ctx, tc, I_d, Q_d, out_d, band_d)
    nc.compile()
    return nc


def build_kernel(ctx, tc, I_d, Q_d, out_d, band_d):
    nc = tc.nc
    FW = NB * HW_    # 2048 free cols per quantity-image
    QW = NQ * FW     # 8192 free cols for the 4 stacked quantities
    HF = FW // 2     # 1024

    pQin = ctx.enter_context(tc.tile_pool(name="qin", bufs=2))
    pIf = ctx.enter_context(tc.tile_pool(name="If", bufs=3))
    consts = ctx.enter_context(tc.tile_pool(name="consts", bufs=1))
    pY = ctx.enter_context(tc.tile_pool(name="ymid", bufs=2))
    pAB = ctx.enter_context(tc.tile_pool(name="ab", bufs=2))
    pOut = ctx.enter_context(tc.tile_pool(name="outp", bufs=2))
    pS = ctx.enter_context(tc.tile_pool(name="stash", bufs=2))
    pS1 = ctx.enter_context(tc.tile_pool(name="chain", bufs=1))
    pV = ctx.enter_context(tc.tile_pool(name="psv", bufs=2, space="PSUM"))
    pQ = ctx.enter_context(tc.tile_pool(name="psq", bufs=1, space="PSUM"))

    # -- PE warmup: ~36 matmuls on a memset tile, no DMA dependencies. ----
    # Bridges the initial input-DMA wait and releases the HAM clock gate
    # before real matmuls arrive.
    wsrc = consts.tile([P, HW_], F16, tag="wsrc", name="wsrc")
    nc.gpsimd.memset(wsrc[:], 0.0)
    for k in range(N_WARMUP_MM):
        ps = pV.tile([P, 2 * HW_], F32, tag="ps")
        for i2 in range(2):
            nc.tensor.matmul(ps[:, i2 * HW_:i2 * HW_ + 256],
                             lhsT=wsrc[:, 0:P], rhs=wsrc[:, 0:256],
                             start=True, stop=True)

    def issue_dma(c, st):
        """Prefetch image c's inputs on the sync HWDGE queue: the guide
        image first (it gates the first V-pass), then p/Ip/II, then the
        f32 guide (only needed at stage C)."""
        Q = pQin.tile([P, QW], F16, tag="Qf16", name="Qf16")
        I_f = pIf.tile([P, FW], F32, tag="If", name="If")
        nc.sync.dma_start(
            Q[:, 0:FW].rearrange("p (hb w) -> p hb w", w=HW_),
            Q_d[c, 0].rearrange("(hb hp) w -> hp hb w", hp=P))
        nc.sync.dma_start(
            Q[:, FW:2 * FW].rearrange("p (hb w) -> p hb w", w=HW_),
            Q_d[c, 1].rearrange("(hb hp) w -> hp hb w", hp=P))
        nc.sync.dma_start(
            Q[:, 2 * FW:QW].rearrange("p (q hb w) -> p q hb w", q=2, w=HW_),
            Q_d[c, 2:4].rearrange("q (hb hp) w -> hp q hb w", hp=P))
        nc.sync.dma_start(
            I_f[:].rearrange("p (hb w) -> p hb w", w=HW_),
            I_d[c].rearrange("(hb hp) w -> hp hb w", hp=P))
        st["Q"], st["I_f"] = Q, I_f

    band = consts.tile_from(band_d)

    def vpass(src, src_off, dst, engines):
        """One banded V-pass: src [h|w] fp16 -> dst [w|h] fp16 (box over
        partition axis + transpose). 16 MMs into two 2-bank PSUM tiles,
        each drained by one [128,1024] copy (engine per-half)."""
        for half in range(2):
            ps = pV.tile([P, 2 * HW_], F32, tag="ps")
            for i2 in range(2):
                i = 2 * half + i2
                for j in range(NB):
                    n0, n1 = _band_range(j)
                    o = src_off + j * HW_ + i * P
                    nc.tensor.matmul(
                        ps[:, i2 * HW_ + n0: i2 * HW_ + n1],
                        lhsT=src[:, o: o + P],
                        rhs=band[:, _BAND_OFF[j]: _BAND_OFF[j] + _BAND_W[j]],
                        start=(j == 0), stop=(j == NB - 1))
            d = dst[:, 2 * half * HW_: 2 * (half + 1) * HW_]
            if engines[half] == "dve":
                nc.vector.tensor_copy(d, ps[:])
            else:
                nc.scalar.copy(d, ps[:])

    def wpass_mm(src, q_tile, j):
        """W-direction banded MMs for output h-chunk j into q_tile."""
        for i in range(NB):
            m0, m1 = _band_range(i)
            nc.tensor.matmul(
                q_tile[:, m0:m1],
                lhsT=src[:, i * HW_ + j * P: i * HW_ + j * P + P],
                rhs=band[:, _BAND_OFF[i]: _BAND_OFF[i] + _BAND_W[i]],
                start=(i == 0), stop=(i == NB - 1))

    def stageA_units(st):
        """4 units: V-passes for I, p, Ip, II from the stacked input."""
        Q = st["Q"]
        for q, ytag in enumerate(("yI", "yp", "yIp", "yII")):
            y = pY.tile([P, FW], F16, tag=ytag, name=ytag)
            st[ytag] = y
            vpass(Q, q * FW, y, ["act", "act"])
            yield

    def stageB_units(st):
        """4 j-units (W-passes + 4 fp16 stash copies + u/v each); the
        batched a/b chain runs in [128,1024] halves folded into the j=1
        and j=3 units."""
        S1 = pS.tile([P, 2 * FW], F16, tag="mImp", name="mImp")
        S2 = pS.tile([P, 2 * FW], F16, tag="cIpII", name="cIpII")

        def mI(s):
            return S1[:, s.start:s.stop]

        def mp(s):
            return S1[:, FW + s.start:FW + s.stop]

        def cIp(s):
            return S2[:, s.start:s.stop]

        def cII(s):
            return S2[:, FW + s.start:FW + s.stop]

        u = pS1.tile([P, FW], F16, tag="u", name="u")
        v = pS1.tile([P, FW], F16, tag="v", name="v")
        cov = pS1.tile([P, FW], F16, tag="cov", name="cov")
        den = pS1.tile([P, FW], F16, tag="den", name="den")
        t = pS1.tile([P, FW], F16, tag="t", name="t")
        a_f = pAB.tile([P, FW], F16, tag="af", name="af")
        b_f = pAB.tile([P, FW], F16, tag="bf", name="bf")
        st["a_f"], st["b_f"] = a_f, b_f

        def chain_half(h):
            # batched chain over j-pair h: fp16 2-tensor ops in DVE 2x mode
            sl = slice(h * 2 * HW_, (h + 1) * 2 * HW_)
            nc.vector.tensor_tensor(cov[:, sl], cIp(sl), u[:, sl],
                                    op=ALU.subtract)                 # DVE 2x
            nc.vector.tensor_tensor(den[:, sl], cII(sl), v[:, sl],
                                    op=ALU.subtract)                 # DVE 2x
            nc.vector._custom_dve(
                _get_mul_recip_op(), out=a_f[:, sl], in0=den[:, sl],
                in1=cov[:, sl], s0=-0.23549792, s1=2.0017324, imm2=EPS)
            nc.vector.tensor_tensor(t[:, sl], mI(sl), a_f[:, sl],
                                    op=ALU.mult)                     # DVE 2x
            nc.vector.tensor_tensor(b_f[:, sl], mp(sl), t[:, sl],
                                    op=ALU.subtract)                 # DVE 2x

        for j in range(NB):
            qI = pQ.tile([P, HW_], F32, tag="qI")
            qp = pQ.tile([P, HW_], F32, tag="qp")
            qIp = pQ.tile([P, HW_], F32, tag="qIp")
            qII = pQ.tile([P, HW_], F32, tag="qII")
            wpass_mm(st["yI"], qI, j)
            wpass_mm(st["yp"], qp, j)
            wpass_mm(st["yIp"], qIp, j)
            wpass_mm(st["yII"], qII, j)
            sl = slice(j * HW_, (j + 1) * HW_)
            # free each q bank with one fp16 stash copy (ACT/DVE only)
            nc.scalar.copy(mI(sl), qI[:])
            nc.scalar.copy(mp(sl), qp[:])
            nc.vector.tensor_copy(cIp(sl), qIp[:])
            nc.vector.tensor_copy(cII(sl), qII[:])
            # u, v per-j on Pool: off the a-chain critical path
            nc.gpsimd.tensor_tensor(u[:, sl], mI(sl), mp(sl), op=ALU.mult)
            nc.gpsimd.tensor_tensor(v[:, sl], mI(sl), mI(sl), op=ALU.mult)
            if j == 1:
                chain_half(0)
            elif j == 3:
                chain_half(1)
            yield

    def stageC_units(c, st):
        """6 units: V(a), V(b), then per-j W-passes + combine + out DMA.
        ra/rb reuse the stage-B q banks (pQ pool)."""
        a_f, b_f, I_f = st["a_f"], st["b_f"], st["I_f"]
        ya = pY.tile([P, FW], F16, tag="ya", name="ya")
        yb = pY.tile([P, FW], F16, tag="yb", name="yb")
        vpass(a_f, 0, ya, ["act", "act"])
        yield
        vpass(b_f, 0, yb, ["act", "act"])
        yield

        out_t = pOut.tile([P, FW], F32, tag="out", name="out")
        rtags = ("qI", "qp") if c % 2 == 0 else ("qIp", "qII")
        for j in range(NB):
            ra = pQ.tile([P, HW_], F32, tag=rtags[0], name="ra")
            rb = pQ.tile([P, HW_], F32, tag=rtags[1], name="rb")
            wpass_mm(ya, ra, j)
            wpass_mm(yb, rb, j)
            sl = slice(j * HW_, (j + 1) * HW_)
            f1 = pS.tile([P, HW_], F32, tag="f1", name="f1")
            nc.vector.scalar_tensor_tensor(
                f1[:], ra[:], 1.0, I_f[:, sl], op0=ALU.mult, op1=ALU.mult)
            nc.vector.scalar_tensor_tensor(
                out_t[:, sl], rb[:], 1.0, f1[:], op0=ALU.mult, op1=ALU.add)
            nc.sync.dma_start(
                out_d[c].rearrange("(hb hp) w -> hp hb w", hp=P)[:, j, :],
                out_t[:, sl])
            yield

    # -- fine-grained software pipeline over the 3 images ------------------
    sts = [{}, {}, {}]
    issue_dma(0, sts[0])
    A = [stageA_units(sts[c]) for c in range(CH)]
    B = [stageB_units(sts[c]) for c in range(CH)]
    C = [stageC_units(c, sts[c]) for c in range(CH)]

    def run(gen):
        next(gen)

    for _ in range(4):
        run(A[0])
    issue_dma(1, sts[1])
    for _ in range(4):
        run(B[0]); run(A[1])
    issue_dma(2, sts[2])
    for _ in range(4):
        run(B[1]); run(A[2])
    run(C[0]); run(B[2])      # C0.va   B2.j0
    run(C[0]); run(B[2])      # C0.vb   B2.j1
    run(C[0]); run(B[2])      # C0.j0   B2.j2
    run(C[0]); run(B[2])      # C0.j1   B2.j3
    run(C[0]); run(C[0])      # C0.j2   C0.j3
    for _ in range(6):
        run(C[1]); run(C[2])


_NC_CACHE = None
LAST_RESULT = None


def _get_model():
    global _NC_CACHE
    if _NC_CACHE is None:
        _NC_CACHE = build_model()
    return _NC_CACHE


def kernel(I, p):
    global LAST_RESULT
    I = np.asarray(I, dtype=np.float32)
    p = np.asarray(p, dtype=np.float32)
    B = I.shape[0]
    assert I.shape == (B, CH, HW_, HW_), I.shape
    nc = _get_model()
    consts = make_consts()
    I_16 = I.astype(np.float16)
    p_16 = p.astype(np.float16)
    Ip_16 = (I_16.astype(np.float32) * p_16.astype(np.float32)).astype(
        np.float16)
    II_16 = (I * I).astype(np.float16)
    Q = np.ascontiguousarray(
        np.stack([I_16, p_16, Ip_16, II_16], axis=2))  # [B, CH, NQ, H, W]
    in_maps = []
    for k in range(NCORES):
        m = {"I": np.ascontiguousarray(I[k]), "Qf16": Q[k]}
        m.update(consts)
        in_maps.append(m)
    kwargs = {}
    if os.environ.get("BASS_TRACE_DIR"):
        kwargs["tmpdir"] = os.environ["BASS_TRACE_DIR"]
    res = run_bass_kernel_spmd(nc, in_maps, core_ids=list(range(NCORES)), **kwargs)
    LAST_RESULT = res
    out = np.stack([res.results[k]["out"] for k in range(NCORES)], axis=0)
    return out.astype(np.float32)


if __name__ == "__main__":
    rng = np.random.default_rng(0)
    I = rng.random((8, CH, HW_, HW_), dtype=np.float32)
    p = rng.random((8, CH, HW_, HW_), dtype=np.float32)
    out = kernel(I, p)
    print("out", out.shape, out.dtype, float(out.mean()))
